# revision 1
# baseline (speedup 1.0000x reference)
"""Trainium2 Bass kernel for dense multi-head self-attention.

Reference computation (fp32):
    xn  = rms_norm(x) * (gamma + 1)          # F.normalize(x) * sqrt(D) * (gamma+1)
    qkv = xn @ w_qkv ; split into q, k, v    # heads H=16, dim_head 64
    out = softmax(q k^T / sqrt(64)) v
    y   = out @ w_out

Sharding (8 cores): data-parallel over batch (2), tensor-parallel over heads
(16 -> 4 groups of 4). Core c handles batch c//4, head group c%4. w_qkv is
column-sliced, w_out row-sliced per head group; each core emits a partial
[2048, 1024] output which the host sums per batch (unshard of the row-parallel
w_out partial-sum layout). No cross-device communication inside the kernel.

Per-core dataflow (all on-chip after the initial loads, transposed layout):
    xT [1024, 2048] (host-transposed slice) -> SBUF as 8 chunks [128, 2048]
    gp1 = gamma+1 folded into w_qkv rows on device (q part also gets the
    1/sqrt(64) softmax scale); rms scale rs[t] = 32/||x_t|| computed via
    ones-matmul over x^2, applied at the psum->sbuf drains of q^T/k^T/v.
    Attention per (q-group of 512, head): S^T chunks [128 k, 512 q] on PE,
    exp on ACT (PSUM->SBUF), O'^T = [V | 1]^T @ expS^T accumulated on PE
    (ones column yields softmax denominators in row 64), normalization by
    1/denom via gpsimd partition_broadcast + reciprocal + one DVE multiply,
    final projection from the transposed attention output.

Dtypes: projections run in float32r (fp32 bits, PE rounds to ~tf32, 2 cyc/row);
attention matmuls (scores and attn@V) optionally in bf16 (1 cyc/row + fast
weight load) — "mixed" mode. Accumulation is always fp32 in PSUM.
"""

import numpy as np

import concourse.bass as bass  # noqa: F401
import concourse.mybir as mybir
import concourse.tile as tile
from concourse import bacc
from concourse.bass_utils import run_bass_kernel_spmd

# Problem constants (hardcoded per contract; kernel.py must be self-contained).
B = 2          # batch
N = 2048       # sequence length
D = 1024       # model dim
H = 16         # total heads
DH = 64        # dim per head
HL = 4         # heads per core
DQ = HL * DH   # 256 = per-core q/k/v width
NCORES = 8

P = 128        # partitions

F32 = mybir.dt.float32
F32R = mybir.dt.float32r
BF16 = mybir.dt.bfloat16


def build_attention_kernel(mode="mixed", n=N, d=D, hl=HL, dh=DH):
    """Build the single-core SPMD Bass program (dims overridable for sim).

    mode: "f32r"  - all matmuls float32r (~tf32 precision, 2 cyc/row)
          "mixed" - projections f32r, attention (S^T, O^T) bf16 (1 cyc/row)
          "bf16"  - all matmuls bf16
    """
    PDT = BF16 if mode == "bf16" else F32R   # projection dtype
    ADT = F32R if mode == "f32r" else BF16   # attention dtype
    dq = hl * dh
    ndc = d // P        # dim chunks of 128
    nt4 = n // 512      # token tiles of 512
    nt16 = n // P       # token tiles of 128
    kc_n = n // P       # key chunks of 128
    qg_n = n // 512     # query groups of 512
    hp_n = hl // 2      # head pairs
    OV_DELAY = 6        # chunks the attn@V matmul lags behind exp
    scale = dh ** -0.5

    # Bacc: its finalize() runs the TRN2 lowering passes (register alloc,
    # wait splitting via generate_event_semaphores, act table loads).
    nc = bacc.Bacc()
    xT_d = nc.declare_dram_parameter("xT", [d, n], PDT, isOutput=False)
    wqkv_d = nc.declare_dram_parameter("wqkv", [d, 3 * dq], PDT, isOutput=False)
    wout_d = nc.declare_dram_parameter("wout", [dq, d], PDT, isOutput=False)
    gamma_d = nc.declare_dram_parameter("gamma", [d], F32, isOutput=False)
    out_d = nc.declare_dram_parameter("out", [n, d], F32, isOutput=True)

    kc2_n = dq // P     # contraction chunks for the output projection
    on_n = d // 512     # output-column tiles
    n_halves = 4 if n >= 2048 else (2 if n >= 1024 else 1)
    nh = n // n_halves

    with tile.TileContext(nc) as tc:
        with (
            # 8 KiB/partition slots; holds the xT chunks during the
            # projections, recycled for expS^T tiles during attention.
            tc.tile_pool(name="big", bufs=max(ndc * n_halves, 8)) as big,
            tc.tile_pool(name="xsq", bufs=2) as xsq_pool,
            tc.tile_pool(name="consts", bufs=1) as consts,
            tc.tile_pool(name="weights", bufs=1) as weights,
            tc.tile_pool(name="qkt", bufs=1) as qkt,
            tc.tile_pool(name="vpool", bufs=1) as vpool,
            tc.tile_pool(name="recip", bufs=2) as recip,
            tc.tile_pool(name="aot", bufs=2) as aot_pool,
            tc.tile_pool(name="outsb", bufs=3) as outsb,
            tc.tile_pool(name="st_ps", bufs=2, space="PSUM") as st_ps,
            tc.tile_pool(name="ot_ps", bufs=2, space="PSUM") as ot_ps,
            tc.tile_pool(name="proj_ps", bufs=2, space="PSUM") as proj_ps,
            tc.tile_pool(name="dram", bufs=2, space="DRAM") as dram_pool,
        ):
            # Weights/gamma first so the fold overlaps the big x load.
            g_sb = consts.tile([P, ndc], F32, tag="gamma")
            nc.sync.dma_start(
                out=g_sb, in_=gamma_d[:].rearrange("(dc p) -> p dc", p=P)
            )
            wqkv_sb = weights.tile([P, ndc, 3 * dq], PDT, tag="wqkv")
            nc.sync.dma_start(
                out=wqkv_sb, in_=wqkv_d[:].rearrange("(dc p) c -> p dc c", p=P)
            )
            wout_sb = weights.tile([P, kc2_n, d], PDT, tag="wout")
            nc.sync.dma_start(
                out=wout_sb, in_=wout_d[:].rearrange("(kc p) c -> p kc c", p=P)
            )
            xT = xT_d[:].rearrange("(dc p) (h t) -> dc h p t", p=P, h=n_halves)
            xt_sb = [[None] * n_halves for _ in range(ndc)]
            for h2 in range(n_halves):
                for dc in range(ndc):
                    t = big.tile([P, nh], PDT, tag="big", name=f"xt{dc}_{h2}")
                    nc.sync.dma_start(out=t, in_=xT[dc, h2])
                    xt_sb[dc][h2] = t

            def xt_slice(dc, lo, size):
                h2 = lo // nh
                assert lo // nh == (lo + size - 1) // nh
                return xt_sb[dc][h2][:, lo - h2 * nh : lo - h2 * nh + size]

            # gp1 = gamma + 1 (for k/v columns); gp1q additionally folds the
            # 1/sqrt(dh) attention scale into the q columns.
            gp1 = consts.tile([P, ndc], F32, tag="gp1")
            nc.vector.tensor_scalar_add(out=gp1, in0=g_sb, scalar1=1.0)
            gp1q = consts.tile([P, ndc], F32, tag="gp1q")
            nc.vector.tensor_scalar(
                out=gp1q,
                in0=g_sb,
                scalar1=1.0,
                scalar2=scale,
                op0=mybir.AluOpType.add,
                op1=mybir.AluOpType.mult,
            )
            for dc in range(ndc):
                nc.vector.tensor_scalar_mul(
                    out=wqkv_sb[:, dc, 0:dq],
                    in0=wqkv_sb[:, dc, 0:dq],
                    scalar1=gp1q[:, dc : dc + 1],
                )
                nc.vector.tensor_scalar_mul(
                    out=wqkv_sb[:, dc, dq : 3 * dq],
                    in0=wqkv_sb[:, dc, dq : 3 * dq],
                    scalar1=gp1[:, dc : dc + 1],
                )

            # rs[t] = sqrt(d) / ||x_t||, via ones-matmul over x^2 (reduction
            # over the partition dim), sqrt on ACT, reciprocal on DVE.
            # memset can't target float32r; memset an f32 scratch and
            # cast-copy (rounding) into the tiles that feed the PE.
            ones_f32 = consts.tile([P, nt16 * hl], F32, tag="ones_f32")
            nc.vector.memset(ones_f32, 1.0)
            ones_col = consts.tile([P, 1], PDT, tag="ones")
            nc.vector.tensor_copy(ones_col, ones_f32[:, 0:1])
            # sumsq/rs per token-half so the rms scale for half 0 is ready
            # while half 1 is still loading. Reciprocal cost ~7.5ns per FREE
            # element (partition-independent), so take it in column form via
            # two SBUF->SBUF transpose DMAs.
            norm2 = consts.tile([1, n], F32, tag="norm2")
            rs_bcast = consts.tile([P, n], F32, tag="rs_bcast")
            rs_col = consts.tile([P, nt16], F32, tag="rs_col")
            nch = nt16 // n_halves  # rs_col columns per half
            for h2 in range(n_halves):
                ssh = st_ps.tile([1, 1024], F32, tag="st", name=f"ss{h2}")
                for dc in range(ndc):
                    sq = xsq_pool.tile([P, nh], PDT, tag="xsq", name=f"sq{h2}_{dc}")
                    nc.scalar.square(sq, xt_sb[dc][h2])
                    for i in range(nh // 512):
                        nc.tensor.matmul(
                            ssh[:, i * 512 : (i + 1) * 512],
                            lhsT=ones_col,
                            rhs=sq[:, i * 512 : (i + 1) * 512],
                            start=(dc == 0),
                            stop=(dc == ndc - 1),
                        )
                n2h = norm2[:, h2 * nh : (h2 + 1) * nh]
                # sqrt(sumsq / d) = ||x|| / sqrt(d) -> reciprocal gives rs.
                nc.scalar.activation(
                    out=n2h,
                    in_=ssh[:, 0:nh] if nh <= 1024 else ssh,
                    func=mybir.ActivationFunctionType.Sqrt,
                    scale=1.0 / d,
                )
                rrowh = consts.tile([1, nh], F32, tag=f"rrow{h2}", name=f"rrow{h2}")
                nc.vector.reciprocal(rrowh, n2h)
                nc.gpsimd.partition_broadcast(
                    rs_bcast[:, h2 * nh : (h2 + 1) * nh], rrowh
                )
                # rs in column form feeds only the (late) v drains: DRAM hop.
                rs_dh = dram_pool.tile([nh], F32, tag="rs_dh", name=f"rsd{h2}")
                nc.sync.dma_start(out=rs_dh, in_=rrowh)
                nc.sync.dma_start(
                    out=rs_col[:, h2 * nch : (h2 + 1) * nch],
                    in_=rs_dh.rearrange("(t p) -> p t", p=P),
                )


            # q^T / k^T projections: [128 rows = head-pair x 64 dims, tokens].
            # rms normalization (rs per token) applied at the psum drain.
            qT = qkt.tile([P, hp_n, n], ADT, tag="qT")
            kT = qkt.tile([P, hp_n, n], ADT, tag="kT")
            for h2 in range(n_halves):
                for hp in range(hp_n):
                    for part in range(2):  # 0 = q, 1 = k
                        for nt in range(h2 * nt4 // n_halves, (h2 + 1) * nt4 // n_halves):
                            ps = proj_ps.tile([P, 512], F32, tag="proj", name="psqk")
                            off = part * dq + hp * P
                            for dc in range(ndc):
                                nc.tensor.matmul(
                                    ps,
                                    lhsT=wqkv_sb[:, dc, off : off + P],
                                    rhs=xt_slice(dc, nt * 512, 512),
                                    start=(dc == 0),
                                    stop=(dc == ndc - 1),
                                )
                            dst = qT if part == 0 else kT
                            nc.vector.tensor_mul(
                                out=dst[:, hp, nt * 512 : (nt + 1) * 512],
                                in0=ps,
                                in1=rs_bcast[:, nt * 512 : (nt + 1) * 512],
                            )

            # v projection in natural orientation [token, head*dh], with a
            # ones column appended per head (softmax denominator trick).
            v_sb = vpool.tile([P, nt16, hl, dh + 1], ADT, tag="v")
            nc.vector.tensor_copy(
                v_sb[:, :, :, dh : dh + 1].rearrange("p a b o -> p (a b o)"),
                ones_f32,
            )
            for ntt in range(nt16):
                ps = proj_ps.tile([P, dq], F32, tag="proj", name="psv")
                for dc in range(ndc):
                    nc.tensor.matmul(
                        ps,
                        lhsT=xt_slice(dc, ntt * P, P),
                        rhs=wqkv_sb[:, dc, 2 * dq : 3 * dq],
                        start=(dc == 0),
                        stop=(dc == ndc - 1),
                    )
                nc.vector.tensor_scalar_mul(
                    out=v_sb[:, ntt, :, 0:dh],
                    in0=ps.rearrange("p (h dd) -> p h dd", h=hl),
                    scalar1=rs_col[:, ntt : ntt + 1],
                )

            # Attention + output projection, one query group (512) at a
            # time. Each qg's output projection is emitted in the middle of
            # the NEXT qg's attention stream so the PE never head-blocks on
            # the softmax-normalize latency.
            out_ap = out_d[:]
            pending_outproj = []
            pending_norm = []

            def emit_norm(ots, aot, hp):
                for sub in range(2):
                    # 1/denom on DVE (idle during attention; Ln on ACT would
                    # thrash activation table sets against the scores' Exp).
                    # Deferred into the NEXT iteration's stream, so the 3.3us
                    # reciprocal latency is off every engine's critical path.
                    rr = recip.tile([1, 512], F32, tag="rrow", name="rr")
                    nc.vector.reciprocal(rr, ots[sub][dh : dh + 1, :])
                    rb = recip.tile([dh, 512], F32, tag="rbcast", name="rb")
                    nc.gpsimd.partition_broadcast(rb, rr, channels=dh)
                    nc.vector.tensor_mul(
                        out=aot[sub * dh : (sub + 1) * dh, hp, :],
                        in0=ots[sub][0:dh, :],
                        in1=rb,
                    )

            def emit_outproj(qg, aot):
                for j in range(4):
                    ntt = qg * 4 + j
                    for on in range(on_n):
                        ps = proj_ps.tile([P, 512], F32, tag="proj", name="pso")
                        for kc2 in range(kc2_n):
                            nc.tensor.matmul(
                                ps,
                                lhsT=aot[:, kc2, j * P : (j + 1) * P],
                                rhs=wout_sb[:, kc2, on * 512 : (on + 1) * 512],
                                start=(kc2 == 0),
                                stop=(kc2 == kc2_n - 1),
                            )
                        ob = outsb.tile([P, 512], F32, tag="outsb", name="ob")
                        nc.vector.tensor_copy(ob, ps)
                        nc.sync.dma_start(
                            out=out_ap[ntt * P : (ntt + 1) * P, on * 512 : (on + 1) * 512],
                            in_=ob,
                        )

            for qg in range(qg_n):
                qs = slice(qg * 512, (qg + 1) * 512)
                aot = aot_pool.tile([P, kc2_n, 512], PDT, tag="aot", name=f"aot{qg}")
                for hp in range(hp_n):
                    ots = [
                        ot_ps.tile([dh + 1, 512], F32, tag="ot", name=f"ot{qg}_{hp}_{s}")
                        for s in range(2)
                    ]
                    # Software-pipeline: OV lags S^T/exp by OV_DELAY chunks
                    # so the in-order PE queue never head-blocks on the ot
                    # psum slots while the previous iteration normalizes.
                    # S^T psum tiles hold a kc PAIR ([128, 1024], 2 banks) so
                    # each ACT exp op covers 1024 elements (halved op count).
                    def do_ov(kc, ests, half):
                        for sub in range(2):
                            nc.tensor.matmul(
                                ots[sub],
                                lhsT=v_sb[:, kc, hp * 2 + sub, :],
                                rhs=ests[sub][:, half * 512 : (half + 1) * 512],
                                start=(kc == 0),
                                stop=(kc == kc_n - 1),
                            )

                    ov_q = []
                    for kcp in range(kc_n // 2):
                        ests = []
                        for sub in range(2):
                            est = big.tile(
                                [P, 1024], ADT, tag="big", name=f"est{qg}_{hp}_{kcp}_{sub}"
                            )
                            stp = st_ps.tile([P, 1024], F32, tag="st", name="stp")
                            for half in range(2):
                                kc = kcp * 2 + half
                                # S^T chunk [128 keys, 512 queries] (K=64).
                                nc.tensor.matmul(
                                    stp[:, half * 512 : (half + 1) * 512],
                                    lhsT=kT[sub * dh : (sub + 1) * dh, hp, kc * P : (kc + 1) * P],
                                    rhs=qT[sub * dh : (sub + 1) * dh, hp, qs],
                                    start=True,
                                    stop=True,
                                    tile_position=(sub * dh, 0),
                                )
                            nc.scalar.activation(
                                out=est,
                                in_=stp,
                                func=mybir.ActivationFunctionType.Exp,
                            )
                            ests.append(est)
                        for half in range(2):
                            ov_q.append((kcp * 2 + half, ests, half))
                        while len(ov_q) > OV_DELAY:
                            do_ov(*ov_q.pop(0))
                        if kcp == 1 and pending_norm:
                            emit_norm(*pending_norm.pop(0))
                        if hp == 0 and kcp == 5 and pending_outproj:
                            emit_outproj(*pending_outproj.pop(0))
                    for item in ov_q:
                        do_ov(*item)
                    pending_norm.append((ots, aot, hp))
                pending_outproj.append((qg, aot))
            for item in pending_norm:
                emit_norm(*item)
            for item in pending_outproj:
                emit_outproj(*item)
    nc.finalize()
    return nc


_NC_CACHE = {}


def _get_nc(mode="mixed"):
    if mode not in _NC_CACHE:
        _NC_CACHE[mode] = build_attention_kernel(mode)
    return _NC_CACHE[mode]


def shard_inputs(x, gamma, w_qkv, w_out, mode="mixed"):
    """FULL inputs -> list of 8 per-core input maps."""
    import ml_dtypes

    pdt = ml_dtypes.bfloat16 if mode == "bf16" else np.float32
    d = x.shape[-1]
    dq = w_out.shape[0] // 4
    in_maps = []
    for c in range(NCORES):
        bi, g = c // 4, c % 4
        cs = slice(g * dq, (g + 1) * dq)
        wqkv_s = np.concatenate(
            [w_qkv[:, cs], w_qkv[:, d:][:, cs], w_qkv[:, 2 * d:][:, cs]], axis=1
        )
        in_maps.append(
            {
                "xT": np.ascontiguousarray(x[bi].T).astype(pdt),
                "wqkv": np.ascontiguousarray(wqkv_s).astype(pdt),
                "wout": np.ascontiguousarray(w_out[cs, :]).astype(pdt),
                "gamma": np.ascontiguousarray(gamma).astype(np.float32),
            }
        )
    return in_maps


def unshard_outputs(results):
    """8 partial [N, D] outputs -> full [B, N, D] (sum head groups per batch)."""
    outs = [r["out"] for r in results]
    return np.stack(
        [
            outs[0] + outs[1] + outs[2] + outs[3],
            outs[4] + outs[5] + outs[6] + outs[7],
        ]
    ).astype(np.float32)


def run(x, gamma, w_qkv, w_out, mode="mixed", **spmd_kwargs):
    nc = _get_nc(mode)
    in_maps = shard_inputs(x, gamma, w_qkv, w_out, mode)
    res = run_bass_kernel_spmd(nc, in_maps, list(range(NCORES)), **spmd_kwargs)
    return unshard_outputs(res.results), res


def kernel(x, gamma, w_qkv, w_out):
    out, _ = run(
        np.asarray(x), np.asarray(gamma), np.asarray(w_qkv), np.asarray(w_out)
    )
    return out

